# revision 17
# baseline (speedup 1.0000x reference)
"""Multi-head attention (B=4, L=2048, C=1024, H=16, HD=64) on 8 NeuronCores.

Sharding: tensor-parallel over heads — 2 heads per core. Each core computes
its heads' QKV projection, attention, and a partial output projection over
its 128 ctx channels; the host sums the 8 partial outputs.

Per-core kernel layout notes:
  - All projections/attention keep "T" layouts (channels on partitions) so
    every matmul contraction runs over the partition dim with 512-wide
    moving operands (float32r fast path; exp-probabilities side in bf16).
  - Softmax skips the max-subtraction (scores are ~N(0, 1/9): exp is safe)
    and normalizes after the ctx matmul using a ones-column appended to v
    (rowsum rides along as PSUM partition 64 of the ctx accumulation).
  - Matmuls of the same shape are batched into runs (PE shape switches
    measured ~0.9us each on TRN2).
"""

import numpy as np
import ml_dtypes

import concourse.bass as bass
import concourse.mybir as mybir
import concourse.tile as tile
from concourse import bacc
from concourse.bass_utils import run_bass_kernel_spmd

B, L, C, H, HD = 4, 2048, 1024, 16, 64
NCORES = 8
HPC = H // NCORES  # heads per core = 2
F32 = mybir.dt.float32
F32R = mybir.dt.float32r
BF16 = mybir.dt.bfloat16
EXP = mybir.ActivationFunctionType.Exp

LCHUNK = 512          # token chunk for moving operands
NLC = L // LCHUNK     # 4
NKT = L // 128        # 16 k tiles per sequence
NCT = C // 128        # 8 contraction tiles for the projections


def build_kernel():
    nc = bacc.Bacc("TRN2", target_bir_lowering=False, debug=False,
                   num_devices=NCORES)

    xT = nc.dram_tensor("xT", [B, C, L], F32R, kind="ExternalInput")
    # wqkv[ci, j] = [128 c, 128 f] tile; j in (0=q both heads, 1=k, 2=v)
    wqkv = nc.dram_tensor("wqkv", [NCT, 3, 128, 128], F32R, kind="ExternalInput")
    bqkv = nc.dram_tensor("bqkv", [3, 128, 1], F32, kind="ExternalInput")
    # wo2: [128 c(2 heads), 1024 o]
    wo2 = nc.dram_tensor("wo2", [128, C], F32R, kind="ExternalInput")
    bo8 = nc.dram_tensor("bo8", [128, C], F32, kind="ExternalInput")
    ident_d = nc.dram_tensor("ident_d", [128, 128], F32R, kind="ExternalInput")
    onesb_d = nc.dram_tensor("onesb_d", [128, 8], BF16, kind="ExternalInput")
    out = nc.dram_tensor("out", [B * L, C], F32, kind="ExternalOutput")

    with tile.TileContext(nc) as tc:
        kernel_body(nc, tc, xT, wqkv, bqkv, wo2, bo8, ident_d,
                    onesb_d, out)
    nc.compile()
    return nc


def kernel_body(nc, tc, xT, wqkv, bqkv, wo2, bo8, ident_d, onesb_d,
                out):
    from contextlib import ExitStack
    ctx = ExitStack()
    with ctx:
        consts = ctx.enter_context(tc.tile_pool(name="consts", bufs=1))
        xpool = ctx.enter_context(tc.tile_pool(name="xpool", bufs=20))
        qkvpool = ctx.enter_context(tc.tile_pool(name="qkvpool", bufs=2))
        vppool = ctx.enter_context(tc.tile_pool(name="vppool", bufs=34))
        epool = ctx.enter_context(tc.tile_pool(name="epool", bufs=18))
        cpool = ctx.enter_context(tc.tile_pool(name="cpool", bufs=2))
        spool = ctx.enter_context(tc.tile_pool(name="spool", bufs=3))
        opool = ctx.enter_context(tc.tile_pool(name="opool", bufs=4))
        # PSUM banks: s-tiles 2x2 + cacc 2 + general 2 = 8
        spsum = ctx.enter_context(tc.tile_pool(name="spsum", bufs=2,
                                               space="PSUM"))
        cpsum = ctx.enter_context(tc.tile_pool(name="cpsum", bufs=2,
                                               space="PSUM"))
        gpsum = ctx.enter_context(tc.tile_pool(name="gpsum", bufs=2,
                                               space="PSUM"))

        # ---- constants ----
        w_tiles = []
        for ci in range(NCT):
            row = []
            for j in range(3):
                t = consts.tile([128, 128], F32R, tag=f"w{ci}_{j}")
                nc.sync.dma_start(out=t, in_=wqkv[ci, j])
                row.append(t)
            w_tiles.append(row)
        b_tiles = []
        for j in range(3):
            t = consts.tile([128, 1], F32, tag=f"b{j}")
            nc.sync.dma_start(out=t, in_=bqkv[j])
            b_tiles.append(t)
        wo_t = consts.tile([128, C], F32R, tag="wo_t")
        nc.sync.dma_start(out=wo_t, in_=wo2[:])
        bias_bc = consts.tile([128, C], F32, tag="bias_bc")
        nc.sync.dma_start(out=bias_bc, in_=bo8[:])
        ident = consts.tile([128, 128], F32R, tag="ident")
        nc.sync.dma_start(out=ident, in_=ident_d[:])
        onesb = consts.tile([128, 8], BF16, tag="onesb")
        nc.sync.dma_start(out=onesb, in_=onesb_d[:])

        # ---- phase helpers (emitted in software-pipelined order below) ----
        def emit_qkv_block(b, lc, qkvT):
            ls = bass.ts(lc, LCHUNK)
            xts = []
            for ci in range(NCT):
                xt = xpool.tile([128, LCHUNK], F32R, tag="xt", name="xt")
                nc.sync.dma_start(out=xt, in_=xT[b, bass.ts(ci, 128), ls])
                xts.append(xt)
            for j in range(3):
                p = gpsum.tile([128, LCHUNK], F32, tag="gpb", name="p")
                for ci in range(NCT):
                    nc.tensor.matmul(p, w_tiles[ci][j][:], xts[ci][:],
                                     start=(ci == 0), stop=(ci == NCT - 1))
                # PSUM -> SBUF with per-partition bias add
                nc.vector.tensor_scalar_add(qkvT[j][:, ls], p, b_tiles[j][:])

        def emit_vplus(qkvT):
            # v -> token-major bf16 tiles [128 l, v_h0 | 1 | v_h1 | 1]
            vplus = []
            for t in range(NKT):
                tp = gpsum.tile([128, 128], F32R, tag="gpb", name="tp")
                nc.tensor.transpose(tp, qkvT[2][:, bass.ts(t, 128)], ident[:])
                vp = vppool.tile([128, 2 * HD + 2], BF16, tag="vp", name="vp")
                nc.vector.tensor_copy(vp[:, 0:HD], tp[:, 0:HD])
                nc.vector.tensor_copy(vp[:, HD + 1:2 * HD + 1], tp[:, HD:2 * HD])
                nc.vector.tensor_copy(vp[:, HD:HD + 1], onesb[:, 0:1])
                nc.vector.tensor_copy(vp[:, 2 * HD + 1:2 * HD + 2], onesb[:, 0:1])
                vplus.append(vp)
            return vplus

        def emit_attn_chunk(h, qc, qkvT, vplus, ctxT2):
            # one head, one 1024-wide q chunk
            hb = h * HD
            vsl = slice(h * (HD + 1), (h + 1) * (HD + 1))
            q0 = qc * 1024
            caccs = [cpsum.tile([HD + 1, LCHUNK], F32, tag="cpb",
                                name=f"cacc{half}")
                     for half in range(2)]
            evec = []
            for i in range(NKT):
                s = spsum.tile([128, 2 * LCHUNK], F32, tag="spb", name="s")
                for half in range(2):
                    nc.tensor.matmul(
                        s[:, bass.ts(half, LCHUNK)],
                        qkvT[1][hb:hb + HD, bass.ts(i, 128)],
                        qkvT[0][hb:hb + HD,
                                bass.ds(q0 + half * LCHUNK, LCHUNK)],
                        start=True, stop=True)
                e = epool.tile([128, 2 * LCHUNK], BF16, tag="e", name="e")
                nc.scalar.activation(e, s, EXP, scale=0.125)
                evec.append(e)
            for i in range(NKT):
                for half in range(2):
                    nc.tensor.matmul(
                        caccs[half],
                        vplus[i][:, vsl],
                        evec[i][:, bass.ts(half, LCHUNK)],
                        start=(i == 0), stop=(i == NKT - 1))
            # normalize: copy cacc out early (frees the PSUM bank),
            # recip rowsum onto partition 0, GpSimd-broadcast, mul
            for half in range(2):
                cacc = caccs[half]
                qs = bass.ds(q0 + half * LCHUNK, LCHUNK)
                csb = spool.tile([HD + 1, LCHUNK], F32, tag="csb", name="csb")
                nc.vector.tensor_copy(csb, cacc)
                rt0 = spool.tile([1, LCHUNK], F32, tag="rt0", name="rt0")
                nc.vector.reciprocal(rt0[0:1, :], csb[HD:HD + 1, :])
                zs = spool.tile([HD, LCHUNK], F32, tag="zs", name="zs")
                nc.gpsimd.partition_broadcast(zs[0:HD, :], rt0[0:1, :])
                nc.vector.tensor_mul(ctxT2[hb:hb + HD, qs],
                                     csb[0:HD, :], zs)

        def emit_outproj(b, ctxT2):
            for t in range(NKT):
                rows = bass.ds(b * L + t * 128, 128)
                for oc in range(C // 512):
                    os_ = bass.ts(oc, 512)
                    o = gpsum.tile([128, 512], F32, tag="gpb", name="o")
                    nc.tensor.matmul(o, ctxT2[:, bass.ts(t, 128)],
                                     wo_t[:, os_], start=True, stop=True)
                    ot = opool.tile([128, 512], F32, tag="ot", name="ot")
                    nc.vector.tensor_add(ot, o, bias_bc[:, os_])
                    nc.sync.dma_start(out=out[rows, os_], in_=ot)

        # ---- software-pipelined emission ----
        # Interleave next batch's qkv blocks between attention chunks so the
        # PE's in-order queue always has dense, ready work behind any stall.
        def new_qkvT():
            return [qkvpool.tile([128, L], F32R, tag=f"qkvT{j}",
                                 name=f"qkvT{j}") for j in range(3)]

        qkvT = new_qkvT()
        for lc in range(NLC):
            emit_qkv_block(0, lc, qkvT)
        vplus = emit_vplus(qkvT)
        for b in range(B):
            ctxT2 = cpool.tile([128, L], F32R, tag="ctxT2", name="ctxT2")
            nxt = new_qkvT() if b + 1 < B else None
            chunks = [(h, qc) for h in range(HPC) for qc in range(2)]
            for k, (h, qc) in enumerate(chunks):
                emit_attn_chunk(h, qc, qkvT, vplus, ctxT2)
                if nxt is not None:
                    emit_qkv_block(b + 1, k, nxt)
            if nxt is not None:
                nxt_vplus = emit_vplus(nxt)
            emit_outproj(b, ctxT2)
            if nxt is not None:
                qkvT, vplus = nxt, nxt_vplus


_NC_CACHE = None


def get_nc():
    global _NC_CACHE
    if _NC_CACHE is None:
        _NC_CACHE = build_kernel()
    return _NC_CACHE


def prepare_in_maps(x, W_qkv, b_qkv, W_out, b_out):
    x = np.ascontiguousarray(np.asarray(x, np.float32))
    W_qkv = np.asarray(W_qkv, np.float32)
    b_qkv = np.asarray(b_qkv, np.float32)
    W_out = np.asarray(W_out, np.float32)
    b_out = np.asarray(b_out, np.float32)

    xT = np.ascontiguousarray(x.transpose(0, 2, 1))  # [B, C, L]

    in_maps = []
    for core in range(NCORES):
        h0 = HPC * core
        # per-head channel rows in W_qkv: q = h*192..+64, k = +64, v = +128
        qrows = [np.arange(h * 192, h * 192 + 64) for h in (h0, h0 + 1)]
        krows = [q + 64 for q in qrows]
        vrows = [q + 128 for q in qrows]
        fq = np.concatenate(qrows)
        fk = np.concatenate(krows)
        fv = np.concatenate(vrows)
        # wqkv tiles: [ci, j, 128 c, 128 f]
        wt = np.empty((NCT, 3, 128, 128), np.float32)
        for j, rows in enumerate((fq, fk, fv)):
            wT = np.ascontiguousarray(W_qkv[rows].T)  # [1024 c, 128 f]
            wt[:, j] = wT.reshape(NCT, 128, 128)
        bq = np.stack([b_qkv[fq], b_qkv[fk], b_qkv[fv]])[..., None]  # [3,128,1]
        # wo2 = [128 c, 1024 o]: rows 0:64 h0 ctx channels, 64:128 h1
        wo2 = np.concatenate([
            np.ascontiguousarray(W_out[:, (h0 + h) * HD:(h0 + h + 1) * HD].T)
            for h in range(HPC)
        ], axis=0)
        bo8 = np.broadcast_to((b_out / NCORES)[None, :], (128, C))
        in_maps.append({
            "xT": xT,
            "wqkv": wt,
            "bqkv": np.ascontiguousarray(bq),
            "wo2": np.ascontiguousarray(wo2, dtype=np.float32),
            "bo8": np.ascontiguousarray(bo8, dtype=np.float32),
            "ident_d": np.eye(128, dtype=np.float32),
            "onesb_d": np.ones((128, 8), ml_dtypes.bfloat16),
        })
    return in_maps


def kernel(x, W_qkv, b_qkv, W_out, b_out):
    in_maps = prepare_in_maps(x, W_qkv, b_qkv, W_out, b_out)
    res = run_bass_kernel_spmd(get_nc(), in_maps, core_ids=list(range(NCORES)))
    acc = np.zeros((B * L, C), np.float64)
    for core_out in res.results:
        acc += core_out["out"]
    return acc.reshape(B, L, C).astype(np.float32)


if __name__ == "__main__":
    rng = np.random.default_rng(0)
    ins = {
        "x": rng.standard_normal((B, L, C)).astype(np.float32),
        "W_qkv": rng.uniform(-1 / 32, 1 / 32, (3 * C, C)).astype(np.float32),
        "b_qkv": rng.uniform(-1 / 32, 1 / 32, (3 * C,)).astype(np.float32),
        "W_out": rng.uniform(-1 / 32, 1 / 32, (C, C)).astype(np.float32),
        "b_out": rng.uniform(-1 / 32, 1 / 32, (C,)).astype(np.float32),
    }
    o = kernel(**ins)
    print(o.shape, o.dtype)


# revision 18
# speedup vs baseline: 1.2611x; 1.2611x over previous
"""Multi-head attention (B=4, L=2048, C=1024, H=16, HD=64) on 8 NeuronCores.

Sharding: tensor-parallel over heads — 2 heads per core. Each core computes
its heads' QKV projection, attention, and a partial output projection over
its 128 ctx channels; the host sums the 8 partial outputs.

Per-core kernel layout notes:
  - All projections/attention keep "T" layouts (channels on partitions) so
    every matmul contraction runs over the partition dim with 512-wide
    moving operands (float32r fast path; exp-probabilities side in bf16).
  - Softmax skips the max-subtraction (scores are ~N(0, 1/9): exp is safe)
    and normalizes after the ctx matmul using a ones-column appended to v
    (rowsum rides along as PSUM partition 64 of the ctx accumulation).
  - Matmuls of the same shape are batched into runs (PE shape switches
    measured ~0.9us each on TRN2).
"""

import numpy as np
import ml_dtypes

import concourse.bass as bass
import concourse.mybir as mybir
import concourse.tile as tile
from concourse import bacc
from concourse.bass_utils import run_bass_kernel_spmd

B, L, C, H, HD = 4, 2048, 1024, 16, 64
NCORES = 8
HPC = H // NCORES  # heads per core = 2
F32 = mybir.dt.float32
F32R = mybir.dt.float32r
BF16 = mybir.dt.bfloat16
EXP = mybir.ActivationFunctionType.Exp

LCHUNK = 512          # token chunk for moving operands
NLC = L // LCHUNK     # 4
NKT = L // 128        # 16 k tiles per sequence
NCT = C // 128        # 8 contraction tiles for the projections


def build_kernel():
    nc = bacc.Bacc("TRN2", target_bir_lowering=False, debug=False,
                   num_devices=NCORES)

    xT = nc.dram_tensor("xT", [B, C, L], F32R, kind="ExternalInput")
    # wqkv[ci, j] = [128 c, 128 f] tile; j in (0=q both heads, 1=k, 2=v)
    wqkv = nc.dram_tensor("wqkv", [NCT, 3, 128, 128], F32R, kind="ExternalInput")
    bqkv = nc.dram_tensor("bqkv", [3, 128, 1], F32, kind="ExternalInput")
    # wo2: [128 c(2 heads), 1024 o]
    wo2 = nc.dram_tensor("wo2", [128, C], F32R, kind="ExternalInput")
    bo8 = nc.dram_tensor("bo8", [128, C], F32, kind="ExternalInput")
    ident_d = nc.dram_tensor("ident_d", [128, 128], F32R, kind="ExternalInput")
    onesb_d = nc.dram_tensor("onesb_d", [128, 8], BF16, kind="ExternalInput")
    out = nc.dram_tensor("out", [B * L, C], F32, kind="ExternalOutput")

    with tile.TileContext(nc) as tc:
        kernel_body(nc, tc, xT, wqkv, bqkv, wo2, bo8, ident_d,
                    onesb_d, out)
    nc.compile()
    return nc


def kernel_body(nc, tc, xT, wqkv, bqkv, wo2, bo8, ident_d, onesb_d,
                out):
    from contextlib import ExitStack
    ctx = ExitStack()
    with ctx:
        consts = ctx.enter_context(tc.tile_pool(name="consts", bufs=1))
        xpool = ctx.enter_context(tc.tile_pool(name="xpool", bufs=16))
        qkvpool = ctx.enter_context(tc.tile_pool(name="qkvpool", bufs=2))
        vppool = ctx.enter_context(tc.tile_pool(name="vppool", bufs=34))
        epool = ctx.enter_context(tc.tile_pool(name="epool", bufs=16))
        cpool = ctx.enter_context(tc.tile_pool(name="cpool", bufs=2))
        spool = ctx.enter_context(tc.tile_pool(name="spool", bufs=2))
        opool = ctx.enter_context(tc.tile_pool(name="opool", bufs=4))
        # PSUM banks: s-tiles 2x2 + cacc 2 + general 2 = 8
        spsum = ctx.enter_context(tc.tile_pool(name="spsum", bufs=2,
                                               space="PSUM"))
        cpsum = ctx.enter_context(tc.tile_pool(name="cpsum", bufs=2,
                                               space="PSUM"))
        gpsum = ctx.enter_context(tc.tile_pool(name="gpsum", bufs=2,
                                               space="PSUM"))

        # ---- constants ----
        w_tiles = []
        for ci in range(NCT):
            row = []
            for j in range(3):
                t = consts.tile([128, 128], F32R, tag=f"w{ci}_{j}")
                nc.sync.dma_start(out=t, in_=wqkv[ci, j])
                row.append(t)
            w_tiles.append(row)
        b_tiles = []
        for j in range(3):
            t = consts.tile([128, 1], F32, tag=f"b{j}")
            nc.sync.dma_start(out=t, in_=bqkv[j])
            b_tiles.append(t)
        wo_t = consts.tile([128, C], F32R, tag="wo_t")
        nc.sync.dma_start(out=wo_t, in_=wo2[:])
        bias_bc = consts.tile([128, C], F32, tag="bias_bc")
        nc.sync.dma_start(out=bias_bc, in_=bo8[:])
        ident = consts.tile([128, 128], F32R, tag="ident")
        nc.sync.dma_start(out=ident, in_=ident_d[:])
        onesb = consts.tile([128, 8], BF16, tag="onesb")
        nc.sync.dma_start(out=onesb, in_=onesb_d[:])

        # ---- phase helpers (emitted in software-pipelined order below) ----
        def emit_qkv_block(b, lc, qkvT):
            ls = bass.ts(lc, LCHUNK)
            xts = []
            for ci in range(NCT):
                xt = xpool.tile([128, LCHUNK], F32R, tag="xt", name="xt")
                nc.sync.dma_start(out=xt, in_=xT[b, bass.ts(ci, 128), ls])
                xts.append(xt)
            for j in range(3):
                p = gpsum.tile([128, LCHUNK], F32, tag="gpb", name="p")
                for ci in range(NCT):
                    nc.tensor.matmul(p, w_tiles[ci][j][:], xts[ci][:],
                                     start=(ci == 0), stop=(ci == NCT - 1))
                # PSUM -> SBUF with per-partition bias add. The q chunk is
                # split per head with the other head's rows zeroed so the
                # scores matmul can run K=128 (K-switches cost ~0.4us).
                if j == 0:
                    q0p, q1p = qkvT[0]
                    nc.vector.tensor_scalar_add(q0p[0:HD, ls], p[0:HD, :],
                                                b_tiles[0][0:HD])
                    nc.vector.tensor_scalar_mul(q0p[HD:128, ls],
                                                p[HD:128, :], 0.0)
                    nc.vector.tensor_scalar_add(q1p[HD:128, ls], p[HD:128, :],
                                                b_tiles[0][HD:128])
                    nc.vector.tensor_scalar_mul(q1p[0:HD, ls],
                                                p[0:HD, :], 0.0)
                else:
                    nc.vector.tensor_scalar_add(qkvT[j][:, ls], p,
                                                b_tiles[j][:])

        def emit_vplus(qkvT):
            # v -> token-major bf16 tiles [128 l, v_h0 | 1 | v_h1 | 1]
            vplus = []
            for t in range(NKT):
                tp = gpsum.tile([128, 128], F32R, tag="gpb", name="tp")
                nc.tensor.transpose(tp, qkvT[2][:, bass.ts(t, 128)], ident[:])
                vp = vppool.tile([128, 2 * HD + 2], BF16, tag="vp", name="vp")
                nc.vector.tensor_copy(vp[:, 0:HD], tp[:, 0:HD])
                nc.vector.tensor_copy(vp[:, HD + 1:2 * HD + 1], tp[:, HD:2 * HD])
                nc.vector.tensor_copy(vp[:, HD:HD + 1], onesb[:, 0:1])
                nc.vector.tensor_copy(vp[:, 2 * HD + 1:2 * HD + 2], onesb[:, 0:1])
                vplus.append(vp)
            return vplus

        def emit_attn_chunk(h, qc, qkvT, vplus, ctxT2):
            # one head, one 1024-wide q chunk
            hb = h * HD
            vsl = slice(h * (HD + 1), (h + 1) * (HD + 1))
            q0 = qc * 1024
            caccs = [cpsum.tile([HD + 1, LCHUNK], F32, tag="cpb",
                                name=f"cacc{half}")
                     for half in range(2)]
            evec = []
            qhp = qkvT[0][h]
            for i in range(NKT):
                s = spsum.tile([128, 2 * LCHUNK], F32, tag="spb", name="s")
                for half in range(2):
                    nc.tensor.matmul(
                        s[:, bass.ts(half, LCHUNK)],
                        qkvT[1][:, bass.ts(i, 128)],
                        qhp[:, bass.ds(q0 + half * LCHUNK, LCHUNK)],
                        start=True, stop=True)
                e = epool.tile([128, 2 * LCHUNK], BF16, tag="e", name="e")
                nc.scalar.activation(e, s, EXP, scale=0.125)
                evec.append(e)
            for i in range(NKT):
                for half in range(2):
                    nc.tensor.matmul(
                        caccs[half],
                        vplus[i][:, vsl],
                        evec[i][:, bass.ts(half, LCHUNK)],
                        start=(i == 0), stop=(i == NKT - 1))
            # normalize: copy cacc out early (frees the PSUM bank),
            # recip rowsum onto partition 0, GpSimd-broadcast, mul
            for half in range(2):
                cacc = caccs[half]
                qs = bass.ds(q0 + half * LCHUNK, LCHUNK)
                csb = spool.tile([HD + 1, LCHUNK], F32, tag="csb", name="csb")
                nc.vector.tensor_copy(csb, cacc)
                rt0 = spool.tile([1, LCHUNK], F32, tag="rt0", name="rt0")
                nc.vector.reciprocal(rt0[0:1, :], csb[HD:HD + 1, :])
                zs = spool.tile([HD, LCHUNK], F32, tag="zs", name="zs")
                nc.gpsimd.partition_broadcast(zs[0:HD, :], rt0[0:1, :])
                nc.vector.tensor_mul(ctxT2[hb:hb + HD, qs],
                                     csb[0:HD, :], zs)

        def emit_outproj(b, ctxT2):
            for t in range(NKT):
                rows = bass.ds(b * L + t * 128, 128)
                for oc in range(C // 512):
                    os_ = bass.ts(oc, 512)
                    o = gpsum.tile([128, 512], F32, tag="gpb", name="o")
                    nc.tensor.matmul(o, ctxT2[:, bass.ts(t, 128)],
                                     wo_t[:, os_], start=True, stop=True)
                    ot = opool.tile([128, 512], F32, tag="ot", name="ot")
                    nc.vector.tensor_add(ot, o, bias_bc[:, os_])
                    nc.sync.dma_start(out=out[rows, os_], in_=ot)

        # ---- software-pipelined emission ----
        # Interleave next batch's qkv blocks between attention chunks so the
        # PE's in-order queue always has dense, ready work behind any stall.
        def new_qkvT():
            qp = tuple(qkvpool.tile([128, L], F32R, tag=f"q{h}p",
                                    name=f"q{h}p") for h in range(HPC))
            kc = qkvpool.tile([128, L], F32R, tag="kc", name="kc")
            vc = qkvpool.tile([128, L], F32R, tag="vc", name="vc")
            return [qp, kc, vc]

        qkvT = new_qkvT()
        for lc in range(NLC):
            emit_qkv_block(0, lc, qkvT)
        vplus = emit_vplus(qkvT)
        for b in range(B):
            ctxT2 = cpool.tile([128, L], F32R, tag="ctxT2", name="ctxT2")
            nxt = new_qkvT() if b + 1 < B else None
            chunks = [(h, qc) for h in range(HPC) for qc in range(2)]
            for k, (h, qc) in enumerate(chunks):
                emit_attn_chunk(h, qc, qkvT, vplus, ctxT2)
                if nxt is not None:
                    emit_qkv_block(b + 1, k, nxt)
            if nxt is not None:
                nxt_vplus = emit_vplus(nxt)
            emit_outproj(b, ctxT2)
            if nxt is not None:
                qkvT, vplus = nxt, nxt_vplus


_NC_CACHE = None


def get_nc():
    global _NC_CACHE
    if _NC_CACHE is None:
        _NC_CACHE = build_kernel()
    return _NC_CACHE


def prepare_in_maps(x, W_qkv, b_qkv, W_out, b_out):
    x = np.ascontiguousarray(np.asarray(x, np.float32))
    W_qkv = np.asarray(W_qkv, np.float32)
    b_qkv = np.asarray(b_qkv, np.float32)
    W_out = np.asarray(W_out, np.float32)
    b_out = np.asarray(b_out, np.float32)

    xT = np.ascontiguousarray(x.transpose(0, 2, 1))  # [B, C, L]

    in_maps = []
    for core in range(NCORES):
        h0 = HPC * core
        # per-head channel rows in W_qkv: q = h*192..+64, k = +64, v = +128
        qrows = [np.arange(h * 192, h * 192 + 64) for h in (h0, h0 + 1)]
        krows = [q + 64 for q in qrows]
        vrows = [q + 128 for q in qrows]
        fq = np.concatenate(qrows)
        fk = np.concatenate(krows)
        fv = np.concatenate(vrows)
        # wqkv tiles: [ci, j, 128 c, 128 f]
        wt = np.empty((NCT, 3, 128, 128), np.float32)
        for j, rows in enumerate((fq, fk, fv)):
            wT = np.ascontiguousarray(W_qkv[rows].T)  # [1024 c, 128 f]
            wt[:, j] = wT.reshape(NCT, 128, 128)
        bq = np.stack([b_qkv[fq], b_qkv[fk], b_qkv[fv]])[..., None]  # [3,128,1]
        # wo2 = [128 c, 1024 o]: rows 0:64 h0 ctx channels, 64:128 h1
        wo2 = np.concatenate([
            np.ascontiguousarray(W_out[:, (h0 + h) * HD:(h0 + h + 1) * HD].T)
            for h in range(HPC)
        ], axis=0)
        bo8 = np.broadcast_to((b_out / NCORES)[None, :], (128, C))
        in_maps.append({
            "xT": xT,
            "wqkv": wt,
            "bqkv": np.ascontiguousarray(bq),
            "wo2": np.ascontiguousarray(wo2, dtype=np.float32),
            "bo8": np.ascontiguousarray(bo8, dtype=np.float32),
            "ident_d": np.eye(128, dtype=np.float32),
            "onesb_d": np.ones((128, 8), ml_dtypes.bfloat16),
        })
    return in_maps


def kernel(x, W_qkv, b_qkv, W_out, b_out):
    in_maps = prepare_in_maps(x, W_qkv, b_qkv, W_out, b_out)
    res = run_bass_kernel_spmd(get_nc(), in_maps, core_ids=list(range(NCORES)))
    acc = np.zeros((B * L, C), np.float64)
    for core_out in res.results:
        acc += core_out["out"]
    return acc.reshape(B, L, C).astype(np.float32)


if __name__ == "__main__":
    rng = np.random.default_rng(0)
    ins = {
        "x": rng.standard_normal((B, L, C)).astype(np.float32),
        "W_qkv": rng.uniform(-1 / 32, 1 / 32, (3 * C, C)).astype(np.float32),
        "b_qkv": rng.uniform(-1 / 32, 1 / 32, (3 * C,)).astype(np.float32),
        "W_out": rng.uniform(-1 / 32, 1 / 32, (C, C)).astype(np.float32),
        "b_out": rng.uniform(-1 / 32, 1 / 32, (C,)).astype(np.float32),
    }
    o = kernel(**ins)
    print(o.shape, o.dtype)


# revision 19
# speedup vs baseline: 1.3207x; 1.0473x over previous
"""Multi-head attention (B=4, L=2048, C=1024, H=16, HD=64) on 8 NeuronCores.

Sharding: tensor-parallel over heads — 2 heads per core. Each core computes
its heads' QKV projection, attention, and a partial output projection over
its 128 ctx channels; the host sums the 8 partial outputs.

Per-core kernel layout notes:
  - All projections/attention keep "T" layouts (channels on partitions) so
    every matmul contraction runs over the partition dim with 512-wide
    moving operands (float32r fast path; exp-probabilities side in bf16).
  - Softmax skips the max-subtraction (scores are ~N(0, 1/9): exp is safe)
    and normalizes after the ctx matmul using a ones-column appended to v
    (rowsum rides along as PSUM partition 64 of the ctx accumulation).
  - Matmuls of the same shape are batched into runs (PE shape switches
    measured ~0.9us each on TRN2).
"""

import numpy as np
import ml_dtypes

import concourse.bass as bass
import concourse.mybir as mybir
import concourse.tile as tile
from concourse import bacc
from concourse.bass_utils import run_bass_kernel_spmd

B, L, C, H, HD = 4, 2048, 1024, 16, 64
NCORES = 8
HPC = H // NCORES  # heads per core = 2
F32 = mybir.dt.float32
F32R = mybir.dt.float32r
BF16 = mybir.dt.bfloat16
EXP = mybir.ActivationFunctionType.Exp
LN = mybir.ActivationFunctionType.Ln

LCHUNK = 512          # token chunk for moving operands
NLC = L // LCHUNK     # 4
NKT = L // 128        # 16 k tiles per sequence
NCT = C // 128        # 8 contraction tiles for the projections


def build_kernel():
    nc = bacc.Bacc("TRN2", target_bir_lowering=False, debug=False,
                   num_devices=NCORES)

    xT = nc.dram_tensor("xT", [B, C, L], F32R, kind="ExternalInput")
    # wqkv[ci, j] = [128 c, 128 f] tile; j in (0=q both heads, 1=k, 2=v)
    wqkv = nc.dram_tensor("wqkv", [NCT, 3, 128, 128], F32R, kind="ExternalInput")
    bqkv = nc.dram_tensor("bqkv", [3, 128, 1], F32, kind="ExternalInput")
    # wo2: [128 c(2 heads), 1024 o]
    wo2 = nc.dram_tensor("wo2", [128, C], F32R, kind="ExternalInput")
    ident_d = nc.dram_tensor("ident_d", [128, 128], F32R, kind="ExternalInput")
    onesb_d = nc.dram_tensor("onesb_d", [128, 8], BF16, kind="ExternalInput")
    out = nc.dram_tensor("out", [B * L, C], F32, kind="ExternalOutput")

    with tile.TileContext(nc) as tc:
        kernel_body(nc, tc, xT, wqkv, bqkv, wo2, ident_d,
                    onesb_d, out)
    nc.compile()
    return nc


def kernel_body(nc, tc, xT, wqkv, bqkv, wo2, ident_d, onesb_d,
                out):
    from contextlib import ExitStack
    ctx = ExitStack()
    with ctx:
        consts = ctx.enter_context(tc.tile_pool(name="consts", bufs=1))
        xpool = ctx.enter_context(tc.tile_pool(name="xpool", bufs=16))
        qkvpool = ctx.enter_context(tc.tile_pool(name="qkvpool", bufs=2))
        vppool = ctx.enter_context(tc.tile_pool(name="vppool", bufs=34))
        epool = ctx.enter_context(tc.tile_pool(name="epool", bufs=16))
        cpool = ctx.enter_context(tc.tile_pool(name="cpool", bufs=2))
        spool = ctx.enter_context(tc.tile_pool(name="spool", bufs=2))
        opool = ctx.enter_context(tc.tile_pool(name="opool", bufs=4))
        # PSUM banks: s-tiles 2x2 + cacc 2 + general 2 = 8
        spsum = ctx.enter_context(tc.tile_pool(name="spsum", bufs=2,
                                               space="PSUM"))
        cpsum = ctx.enter_context(tc.tile_pool(name="cpsum", bufs=2,
                                               space="PSUM"))
        gpsum = ctx.enter_context(tc.tile_pool(name="gpsum", bufs=2,
                                               space="PSUM"))

        # ---- constants ----
        w_tiles = []
        for ci in range(NCT):
            row = []
            for j in range(3):
                t = consts.tile([128, 128], F32R, tag=f"w{ci}_{j}")
                nc.sync.dma_start(out=t, in_=wqkv[ci, j])
                row.append(t)
            w_tiles.append(row)
        b_tiles = []
        for j in range(3):
            t = consts.tile([128, 1], F32, tag=f"b{j}")
            nc.sync.dma_start(out=t, in_=bqkv[j])
            b_tiles.append(t)
        wo_t = consts.tile([128, C], F32R, tag="wo_t")
        nc.sync.dma_start(out=wo_t, in_=wo2[:])
        ident = consts.tile([128, 128], F32R, tag="ident")
        nc.sync.dma_start(out=ident, in_=ident_d[:])
        onesb = consts.tile([128, 8], BF16, tag="onesb")
        nc.sync.dma_start(out=onesb, in_=onesb_d[:])

        # ---- phase helpers (emitted in software-pipelined order below) ----
        def emit_qkv_block(b, lc, qkvT):
            ls = bass.ts(lc, LCHUNK)
            xts = []
            for ci in range(NCT):
                xt = xpool.tile([128, LCHUNK], F32R, tag="xt", name="xt")
                nc.sync.dma_start(out=xt, in_=xT[b, bass.ts(ci, 128), ls])
                xts.append(xt)
            for j in range(3):
                p = gpsum.tile([128, LCHUNK], F32, tag="gpb", name="p")
                for ci in range(NCT):
                    nc.tensor.matmul(p, w_tiles[ci][j][:], xts[ci][:],
                                     start=(ci == 0), stop=(ci == NCT - 1))
                # PSUM -> SBUF with per-partition bias add. The q chunk is
                # split per head with the other head's rows zeroed so the
                # scores matmul can run K=128 (K-switches cost ~0.4us).
                if j == 0:
                    q0p, q1p = qkvT[0]
                    nc.vector.tensor_scalar_add(q0p[0:HD, ls], p[0:HD, :],
                                                b_tiles[0][0:HD])
                    nc.vector.tensor_scalar_mul(q0p[HD:128, ls],
                                                p[HD:128, :], 0.0)
                    nc.vector.tensor_scalar_add(q1p[HD:128, ls], p[HD:128, :],
                                                b_tiles[0][HD:128])
                    nc.vector.tensor_scalar_mul(q1p[0:HD, ls],
                                                p[0:HD, :], 0.0)
                else:
                    nc.vector.tensor_scalar_add(qkvT[j][:, ls], p,
                                                b_tiles[j][:])

        def emit_vplus(qkvT):
            # v -> token-major bf16 tiles [128 l, v_h0 | 1 | v_h1 | 1]
            vplus = []
            for t in range(NKT):
                tp = gpsum.tile([128, 128], F32R, tag="gpb", name="tp")
                nc.tensor.transpose(tp, qkvT[2][:, bass.ts(t, 128)], ident[:])
                vp = vppool.tile([128, 2 * HD + 2], BF16, tag="vp", name="vp")
                nc.vector.tensor_copy(vp[:, 0:HD], tp[:, 0:HD])
                nc.vector.tensor_copy(vp[:, HD + 1:2 * HD + 1], tp[:, HD:2 * HD])
                nc.vector.tensor_copy(vp[:, HD:HD + 1], onesb[:, 0:1])
                nc.vector.tensor_copy(vp[:, 2 * HD + 1:2 * HD + 2], onesb[:, 0:1])
                vplus.append(vp)
            return vplus

        def emit_attn_chunk(h, qc, qkvT, vplus, ctxT2):
            # one head, one 1024-wide q chunk
            hb = h * HD
            vsl = slice(h * (HD + 1), (h + 1) * (HD + 1))
            q0 = qc * 1024
            caccs = [cpsum.tile([HD + 1, LCHUNK], F32, tag="cpb",
                                name=f"cacc{half}")
                     for half in range(2)]
            evec = []
            qhp = qkvT[0][h]
            for i in range(NKT):
                s = spsum.tile([128, 2 * LCHUNK], F32, tag="spb", name="s")
                for half in range(2):
                    nc.tensor.matmul(
                        s[:, bass.ts(half, LCHUNK)],
                        qkvT[1][:, bass.ts(i, 128)],
                        qhp[:, bass.ds(q0 + half * LCHUNK, LCHUNK)],
                        start=True, stop=True)
                e = epool.tile([128, 2 * LCHUNK], BF16, tag="e", name="e")
                nc.scalar.activation(e, s, EXP, scale=0.125)
                evec.append(e)
            for i in range(NKT):
                for half in range(2):
                    nc.tensor.matmul(
                        caccs[half],
                        vplus[i][:, vsl],
                        evec[i][:, bass.ts(half, LCHUNK)],
                        start=(i == 0), stop=(i == NKT - 1))
            # normalize: copy cacc out early (frees the PSUM bank; rowsum
            # row lands on partition 0), 1/Z = exp(-ln Z) on ACT (measured
            # 1.2e-5 rel err), GpSimd-broadcast, multiply on DVE
            for half in range(2):
                cacc = caccs[half]
                qs = bass.ds(q0 + half * LCHUNK, LCHUNK)
                csb = spool.tile([HD, LCHUNK], F32, tag="csb", name="csb")
                nc.vector.tensor_copy(csb, cacc[0:HD, :])
                z0 = spool.tile([1, LCHUNK], F32, tag="z0", name="z0")
                nc.vector.tensor_copy(z0[0:1, :], cacc[HD:HD + 1, :])
                lnz = spool.tile([1, LCHUNK], F32, tag="lnz", name="lnz")
                nc.scalar.activation(lnz[0:1, :], z0[0:1, :], LN)
                rt0 = spool.tile([1, LCHUNK], F32, tag="rt0", name="rt0")
                nc.scalar.activation(rt0[0:1, :], lnz[0:1, :], EXP, scale=-1.0)
                zs = spool.tile([HD, LCHUNK], F32, tag="zs", name="zs")
                nc.gpsimd.partition_broadcast(zs[0:HD, :], rt0[0:1, :])
                nc.vector.tensor_mul(ctxT2[hb:hb + HD, qs],
                                     csb[0:HD, :], zs)

        def emit_outproj(b, ctxT2):
            for t in range(NKT):
                rows = bass.ds(b * L + t * 128, 128)
                for oc in range(C // 512):
                    os_ = bass.ts(oc, 512)
                    o = gpsum.tile([128, 512], F32, tag="gpb", name="o")
                    nc.tensor.matmul(o, ctxT2[:, bass.ts(t, 128)],
                                     wo_t[:, os_], start=True, stop=True)
                    ot = opool.tile([128, 512], F32, tag="ot", name="ot")
                    nc.vector.tensor_copy(ot, o)
                    nc.sync.dma_start(out=out[rows, os_], in_=ot)

        # ---- software-pipelined emission ----
        # Interleave next batch's qkv blocks between attention chunks so the
        # PE's in-order queue always has dense, ready work behind any stall.
        def new_qkvT():
            qp = tuple(qkvpool.tile([128, L], F32R, tag=f"q{h}p",
                                    name=f"q{h}p") for h in range(HPC))
            kc = qkvpool.tile([128, L], F32R, tag="kc", name="kc")
            vc = qkvpool.tile([128, L], F32R, tag="vc", name="vc")
            return [qp, kc, vc]

        qkvT = new_qkvT()
        for lc in range(NLC):
            emit_qkv_block(0, lc, qkvT)
        vplus = emit_vplus(qkvT)
        for b in range(B):
            ctxT2 = cpool.tile([128, L], F32R, tag="ctxT2", name="ctxT2")
            nxt = new_qkvT() if b + 1 < B else None
            chunks = [(h, qc) for h in range(HPC) for qc in range(2)]
            for k, (h, qc) in enumerate(chunks):
                emit_attn_chunk(h, qc, qkvT, vplus, ctxT2)
                if nxt is not None:
                    emit_qkv_block(b + 1, k, nxt)
            if nxt is not None:
                nxt_vplus = emit_vplus(nxt)
            emit_outproj(b, ctxT2)
            if nxt is not None:
                qkvT, vplus = nxt, nxt_vplus


_NC_CACHE = None


def get_nc():
    global _NC_CACHE
    if _NC_CACHE is None:
        _NC_CACHE = build_kernel()
    return _NC_CACHE


def prepare_in_maps(x, W_qkv, b_qkv, W_out, b_out):
    x = np.ascontiguousarray(np.asarray(x, np.float32))
    W_qkv = np.asarray(W_qkv, np.float32)
    b_qkv = np.asarray(b_qkv, np.float32)
    W_out = np.asarray(W_out, np.float32)
    b_out = np.asarray(b_out, np.float32)

    xT = np.ascontiguousarray(x.transpose(0, 2, 1))  # [B, C, L]

    in_maps = []
    for core in range(NCORES):
        h0 = HPC * core
        # per-head channel rows in W_qkv: q = h*192..+64, k = +64, v = +128
        qrows = [np.arange(h * 192, h * 192 + 64) for h in (h0, h0 + 1)]
        krows = [q + 64 for q in qrows]
        vrows = [q + 128 for q in qrows]
        fq = np.concatenate(qrows)
        fk = np.concatenate(krows)
        fv = np.concatenate(vrows)
        # wqkv tiles: [ci, j, 128 c, 128 f]
        wt = np.empty((NCT, 3, 128, 128), np.float32)
        for j, rows in enumerate((fq, fk, fv)):
            wT = np.ascontiguousarray(W_qkv[rows].T)  # [1024 c, 128 f]
            wt[:, j] = wT.reshape(NCT, 128, 128)
        bq = np.stack([b_qkv[fq], b_qkv[fk], b_qkv[fv]])[..., None]  # [3,128,1]
        # wo2 = [128 c, 1024 o]: rows 0:64 h0 ctx channels, 64:128 h1
        wo2 = np.concatenate([
            np.ascontiguousarray(W_out[:, (h0 + h) * HD:(h0 + h + 1) * HD].T)
            for h in range(HPC)
        ], axis=0)
        in_maps.append({
            "xT": xT,
            "wqkv": wt,
            "bqkv": np.ascontiguousarray(bq),
            "wo2": np.ascontiguousarray(wo2, dtype=np.float32),
            "ident_d": np.eye(128, dtype=np.float32),
            "onesb_d": np.ones((128, 8), ml_dtypes.bfloat16),
        })
    return in_maps


def kernel(x, W_qkv, b_qkv, W_out, b_out):
    in_maps = prepare_in_maps(x, W_qkv, b_qkv, W_out, b_out)
    res = run_bass_kernel_spmd(get_nc(), in_maps, core_ids=list(range(NCORES)))
    acc = np.zeros((B * L, C), np.float64)
    for core_out in res.results:
        acc += core_out["out"]
    acc += np.asarray(b_out, np.float64)[None, :]
    return acc.reshape(B, L, C).astype(np.float32)


if __name__ == "__main__":
    rng = np.random.default_rng(0)
    ins = {
        "x": rng.standard_normal((B, L, C)).astype(np.float32),
        "W_qkv": rng.uniform(-1 / 32, 1 / 32, (3 * C, C)).astype(np.float32),
        "b_qkv": rng.uniform(-1 / 32, 1 / 32, (3 * C,)).astype(np.float32),
        "W_out": rng.uniform(-1 / 32, 1 / 32, (C, C)).astype(np.float32),
        "b_out": rng.uniform(-1 / 32, 1 / 32, (C,)).astype(np.float32),
    }
    o = kernel(**ins)
    print(o.shape, o.dtype)


# revision 21
# speedup vs baseline: 1.3245x; 1.0029x over previous
"""Multi-head attention (B=4, L=2048, C=1024, H=16, HD=64) on 8 NeuronCores.

Sharding: tensor-parallel over heads — 2 heads per core. Each core computes
its heads' QKV projection, attention, and a partial output projection over
its 128 ctx channels; the host sums the 8 partial outputs.

Per-core kernel layout notes:
  - All projections/attention keep "T" layouts (channels on partitions) so
    every matmul contraction runs over the partition dim with 512-wide
    moving operands (float32r fast path; exp-probabilities side in bf16).
  - Softmax skips the max-subtraction (scores are ~N(0, 1/9): exp is safe)
    and normalizes after the ctx matmul using a ones-column appended to v
    (rowsum rides along as PSUM partition 64 of the ctx accumulation).
  - Matmuls of the same shape are batched into runs (PE shape switches
    measured ~0.9us each on TRN2).
"""

import numpy as np
import ml_dtypes

import concourse.bass as bass
import concourse.mybir as mybir
import concourse.tile as tile
from concourse import bacc
from concourse.bass_utils import run_bass_kernel_spmd

B, L, C, H, HD = 4, 2048, 1024, 16, 64
NCORES = 8
HPC = H // NCORES  # heads per core = 2
F32 = mybir.dt.float32
F32R = mybir.dt.float32r
BF16 = mybir.dt.bfloat16
EXP = mybir.ActivationFunctionType.Exp
LN = mybir.ActivationFunctionType.Ln

LCHUNK = 512          # token chunk for moving operands
NLC = L // LCHUNK     # 4
NKT = L // 128        # 16 k tiles per sequence
NCT = C // 128        # 8 contraction tiles for the projections


def build_kernel():
    nc = bacc.Bacc("TRN2", target_bir_lowering=False, debug=False,
                   num_devices=NCORES)

    xT = nc.dram_tensor("xT", [B, C, L], F32R, kind="ExternalInput")
    # wqkv[ci, j] = [128 c, 128 f] tile; j in (0=q both heads, 1=k, 2=v)
    wqkv = nc.dram_tensor("wqkv", [NCT, 3, 128, 128], F32R, kind="ExternalInput")
    bqkv = nc.dram_tensor("bqkv", [3, 128, 1], F32, kind="ExternalInput")
    # wo2: [128 c(2 heads), 1024 o]
    wo2 = nc.dram_tensor("wo2", [128, C], F32R, kind="ExternalInput")
    ident_d = nc.dram_tensor("ident_d", [128, 128], F32R, kind="ExternalInput")
    onesb_d = nc.dram_tensor("onesb_d", [128, 8], BF16, kind="ExternalInput")
    out = nc.dram_tensor("out", [B * L, C], F32, kind="ExternalOutput")

    with tile.TileContext(nc) as tc:
        kernel_body(nc, tc, xT, wqkv, bqkv, wo2, ident_d,
                    onesb_d, out)
    nc.compile()
    return nc


def kernel_body(nc, tc, xT, wqkv, bqkv, wo2, ident_d, onesb_d,
                out):
    from contextlib import ExitStack
    ctx = ExitStack()
    with ctx:
        consts = ctx.enter_context(tc.tile_pool(name="consts", bufs=1))
        xpool = ctx.enter_context(tc.tile_pool(name="xpool", bufs=20))
        qkvpool = ctx.enter_context(tc.tile_pool(name="qkvpool", bufs=2))
        vppool = ctx.enter_context(tc.tile_pool(name="vppool", bufs=34))
        epool = ctx.enter_context(tc.tile_pool(name="epool", bufs=16))
        cpool = ctx.enter_context(tc.tile_pool(name="cpool", bufs=2))
        spool = ctx.enter_context(tc.tile_pool(name="spool", bufs=2))
        opool = ctx.enter_context(tc.tile_pool(name="opool", bufs=4))
        # PSUM banks: s-tiles 2x2 + cacc 2 + general 2 = 8
        spsum = ctx.enter_context(tc.tile_pool(name="spsum", bufs=2,
                                               space="PSUM"))
        cpsum = ctx.enter_context(tc.tile_pool(name="cpsum", bufs=2,
                                               space="PSUM"))
        gpsum = ctx.enter_context(tc.tile_pool(name="gpsum", bufs=2,
                                               space="PSUM"))

        # ---- constants ----
        w_tiles = []
        for ci in range(NCT):
            row = []
            for j in range(3):
                t = consts.tile([128, 128], F32R, tag=f"w{ci}_{j}")
                nc.sync.dma_start(out=t, in_=wqkv[ci, j])
                row.append(t)
            w_tiles.append(row)
        b_tiles = []
        for j in range(3):
            t = consts.tile([128, 1], F32, tag=f"b{j}")
            nc.sync.dma_start(out=t, in_=bqkv[j])
            b_tiles.append(t)
        wo_t = consts.tile([128, C], F32R, tag="wo_t")
        nc.sync.dma_start(out=wo_t, in_=wo2[:])
        ident = consts.tile([128, 128], F32R, tag="ident")
        nc.sync.dma_start(out=ident, in_=ident_d[:])
        onesb = consts.tile([128, 8], BF16, tag="onesb")
        nc.sync.dma_start(out=onesb, in_=onesb_d[:])

        # ---- phase helpers (emitted in software-pipelined order below) ----
        def emit_qkv_loads(b, lc):
            ls = bass.ts(lc, LCHUNK)
            xts = []
            for ci in range(NCT):
                xt = xpool.tile([128, LCHUNK], F32R, tag="xt", name="xt")
                nc.sync.dma_start(out=xt, in_=xT[b, bass.ts(ci, 128), ls])
                xts.append(xt)
            return xts

        def emit_qkv_block(b, lc, qkvT, xts):
            ls = bass.ts(lc, LCHUNK)
            for j in range(3):
                p = gpsum.tile([128, LCHUNK], F32, tag="gpb", name="p")
                for ci in range(NCT):
                    nc.tensor.matmul(p, w_tiles[ci][j][:], xts[ci][:],
                                     start=(ci == 0), stop=(ci == NCT - 1))
                # PSUM -> SBUF with per-partition bias add. The q chunk is
                # split per head with the other head's rows zeroed so the
                # scores matmul can run K=128 (K-switches cost ~0.4us).
                if j == 0:
                    q0p, q1p = qkvT[0]
                    nc.vector.tensor_scalar_add(q0p[0:HD, ls], p[0:HD, :],
                                                b_tiles[0][0:HD])
                    nc.vector.tensor_scalar_mul(q0p[HD:128, ls],
                                                p[HD:128, :], 0.0)
                    nc.vector.tensor_scalar_add(q1p[HD:128, ls], p[HD:128, :],
                                                b_tiles[0][HD:128])
                    nc.vector.tensor_scalar_mul(q1p[0:HD, ls],
                                                p[0:HD, :], 0.0)
                else:
                    nc.vector.tensor_scalar_add(qkvT[j][:, ls], p,
                                                b_tiles[j][:])

        def emit_vplus(qkvT):
            # v -> token-major bf16 tiles [128 l, v_h0 | 1 | v_h1 | 1]
            vplus = []
            for t in range(NKT):
                tp = gpsum.tile([128, 128], F32R, tag="gpb", name="tp")
                nc.tensor.transpose(tp, qkvT[2][:, bass.ts(t, 128)], ident[:])
                vp = vppool.tile([128, 2 * HD + 2], BF16, tag="vp", name="vp")
                nc.vector.tensor_copy(vp[:, 0:HD], tp[:, 0:HD])
                nc.vector.tensor_copy(vp[:, HD + 1:2 * HD + 1], tp[:, HD:2 * HD])
                nc.vector.tensor_copy(vp[:, HD:HD + 1], onesb[:, 0:1])
                nc.vector.tensor_copy(vp[:, 2 * HD + 1:2 * HD + 2], onesb[:, 0:1])
                vplus.append(vp)
            return vplus

        def emit_attn_chunk(h, qc, qkvT, vplus, ctxT2):
            # one head, one 1024-wide q chunk
            hb = h * HD
            vsl = slice(h * (HD + 1), (h + 1) * (HD + 1))
            q0 = qc * 1024
            caccs = [cpsum.tile([HD + 1, LCHUNK], F32, tag="cpb",
                                name=f"cacc{half}")
                     for half in range(2)]
            evec = []
            qhp = qkvT[0][h]
            for i in range(NKT):
                s = spsum.tile([128, 2 * LCHUNK], F32, tag="spb", name="s")
                for half in range(2):
                    nc.tensor.matmul(
                        s[:, bass.ts(half, LCHUNK)],
                        qkvT[1][:, bass.ts(i, 128)],
                        qhp[:, bass.ds(q0 + half * LCHUNK, LCHUNK)],
                        start=True, stop=True)
                e = epool.tile([128, 2 * LCHUNK], BF16, tag="e", name="e")
                nc.scalar.activation(e, s, EXP, scale=0.125)
                evec.append(e)
            for i in range(NKT):
                for half in range(2):
                    nc.tensor.matmul(
                        caccs[half],
                        vplus[i][:, vsl],
                        evec[i][:, bass.ts(half, LCHUNK)],
                        start=(i == 0), stop=(i == NKT - 1))
            # normalize: copy cacc out early (frees the PSUM bank; rowsum
            # row lands on partition 0), 1/Z = exp(-ln Z) on ACT (measured
            # 1.2e-5 rel err), GpSimd-broadcast, multiply on DVE
            for half in range(2):
                cacc = caccs[half]
                qs = bass.ds(q0 + half * LCHUNK, LCHUNK)
                csb = spool.tile([HD, LCHUNK], F32, tag="csb", name="csb")
                nc.vector.tensor_copy(csb, cacc[0:HD, :])
                z0 = spool.tile([1, LCHUNK], F32, tag="z0", name="z0")
                nc.vector.tensor_copy(z0[0:1, :], cacc[HD:HD + 1, :])
                lnz = spool.tile([1, LCHUNK], F32, tag="lnz", name="lnz")
                nc.scalar.activation(lnz[0:1, :], z0[0:1, :], LN)
                rt0 = spool.tile([1, LCHUNK], F32, tag="rt0", name="rt0")
                nc.scalar.activation(rt0[0:1, :], lnz[0:1, :], EXP, scale=-1.0)
                zs = spool.tile([HD, LCHUNK], F32, tag="zs", name="zs")
                nc.gpsimd.partition_broadcast(zs[0:HD, :], rt0[0:1, :])
                nc.vector.tensor_mul(ctxT2[hb:hb + HD, qs],
                                     csb[0:HD, :], zs)

        def emit_outproj(b, ctxT2):
            for t in range(NKT):
                rows = bass.ds(b * L + t * 128, 128)
                for oc in range(C // 512):
                    os_ = bass.ts(oc, 512)
                    o = gpsum.tile([128, 512], F32, tag="gpb", name="o")
                    nc.tensor.matmul(o, ctxT2[:, bass.ts(t, 128)],
                                     wo_t[:, os_], start=True, stop=True)
                    ot = opool.tile([128, 512], F32, tag="ot", name="ot")
                    nc.vector.tensor_copy(ot, o)
                    nc.sync.dma_start(out=out[rows, os_], in_=ot)

        # ---- software-pipelined emission ----
        # Interleave next batch's qkv blocks between attention chunks so the
        # PE's in-order queue always has dense, ready work behind any stall.
        def new_qkvT():
            qp = tuple(qkvpool.tile([128, L], F32R, tag=f"q{h}p",
                                    name=f"q{h}p") for h in range(HPC))
            kc = qkvpool.tile([128, L], F32R, tag="kc", name="kc")
            vc = qkvpool.tile([128, L], F32R, tag="vc", name="vc")
            return [qp, kc, vc]

        qkvT = new_qkvT()
        for lc in range(NLC):
            xts = emit_qkv_loads(0, lc)
            emit_qkv_block(0, lc, qkvT, xts)
        vplus = emit_vplus(qkvT)
        for b in range(B):
            ctxT2 = cpool.tile([128, L], F32R, tag="ctxT2", name="ctxT2")
            nxt = new_qkvT() if b + 1 < B else None
            chunks = [(h, qc) for h in range(HPC) for qc in range(2)]
            loads = {}
            if nxt is not None:
                loads[0] = emit_qkv_loads(b + 1, 0)
                loads[1] = emit_qkv_loads(b + 1, 1)
            for k, (h, qc) in enumerate(chunks):
                emit_attn_chunk(h, qc, qkvT, vplus, ctxT2)
                if nxt is not None:
                    if k + 2 < NLC:
                        loads[k + 2] = emit_qkv_loads(b + 1, k + 2)
                    emit_qkv_block(b + 1, k, nxt, loads.pop(k))
            if nxt is not None:
                nxt_vplus = emit_vplus(nxt)
            emit_outproj(b, ctxT2)
            if nxt is not None:
                qkvT, vplus = nxt, nxt_vplus


_NC_CACHE = None


def get_nc():
    global _NC_CACHE
    if _NC_CACHE is None:
        _NC_CACHE = build_kernel()
    return _NC_CACHE


def prepare_in_maps(x, W_qkv, b_qkv, W_out, b_out):
    x = np.ascontiguousarray(np.asarray(x, np.float32))
    W_qkv = np.asarray(W_qkv, np.float32)
    b_qkv = np.asarray(b_qkv, np.float32)
    W_out = np.asarray(W_out, np.float32)
    b_out = np.asarray(b_out, np.float32)

    xT = np.ascontiguousarray(x.transpose(0, 2, 1))  # [B, C, L]

    in_maps = []
    for core in range(NCORES):
        h0 = HPC * core
        # per-head channel rows in W_qkv: q = h*192..+64, k = +64, v = +128
        qrows = [np.arange(h * 192, h * 192 + 64) for h in (h0, h0 + 1)]
        krows = [q + 64 for q in qrows]
        vrows = [q + 128 for q in qrows]
        fq = np.concatenate(qrows)
        fk = np.concatenate(krows)
        fv = np.concatenate(vrows)
        # wqkv tiles: [ci, j, 128 c, 128 f]
        wt = np.empty((NCT, 3, 128, 128), np.float32)
        for j, rows in enumerate((fq, fk, fv)):
            wT = np.ascontiguousarray(W_qkv[rows].T)  # [1024 c, 128 f]
            wt[:, j] = wT.reshape(NCT, 128, 128)
        bq = np.stack([b_qkv[fq], b_qkv[fk], b_qkv[fv]])[..., None]  # [3,128,1]
        # wo2 = [128 c, 1024 o]: rows 0:64 h0 ctx channels, 64:128 h1
        wo2 = np.concatenate([
            np.ascontiguousarray(W_out[:, (h0 + h) * HD:(h0 + h + 1) * HD].T)
            for h in range(HPC)
        ], axis=0)
        in_maps.append({
            "xT": xT,
            "wqkv": wt,
            "bqkv": np.ascontiguousarray(bq),
            "wo2": np.ascontiguousarray(wo2, dtype=np.float32),
            "ident_d": np.eye(128, dtype=np.float32),
            "onesb_d": np.ones((128, 8), ml_dtypes.bfloat16),
        })
    return in_maps


def kernel(x, W_qkv, b_qkv, W_out, b_out):
    in_maps = prepare_in_maps(x, W_qkv, b_qkv, W_out, b_out)
    res = run_bass_kernel_spmd(get_nc(), in_maps, core_ids=list(range(NCORES)))
    acc = np.zeros((B * L, C), np.float64)
    for core_out in res.results:
        acc += core_out["out"]
    acc += np.asarray(b_out, np.float64)[None, :]
    return acc.reshape(B, L, C).astype(np.float32)


if __name__ == "__main__":
    rng = np.random.default_rng(0)
    ins = {
        "x": rng.standard_normal((B, L, C)).astype(np.float32),
        "W_qkv": rng.uniform(-1 / 32, 1 / 32, (3 * C, C)).astype(np.float32),
        "b_qkv": rng.uniform(-1 / 32, 1 / 32, (3 * C,)).astype(np.float32),
        "W_out": rng.uniform(-1 / 32, 1 / 32, (C, C)).astype(np.float32),
        "b_out": rng.uniform(-1 / 32, 1 / 32, (C,)).astype(np.float32),
    }
    o = kernel(**ins)
    print(o.shape, o.dtype)


# revision 27
# speedup vs baseline: 1.3557x; 1.0235x over previous
"""Multi-head attention (B=4, L=2048, C=1024, H=16, HD=64) on 8 NeuronCores.

Sharding: tensor-parallel over heads — 2 heads per core. Each core computes
its heads' QKV projection, attention, and a partial output projection over
its 128 ctx channels; the host sums the 8 partial outputs.

Per-core kernel layout notes:
  - All projections/attention keep "T" layouts (channels on partitions) so
    every matmul contraction runs over the partition dim with 512-wide
    moving operands (float32r fast path; exp-probabilities side in bf16).
  - Softmax skips the max-subtraction (scores are ~N(0, 1/9): exp is safe)
    and normalizes after the ctx matmul using a ones-column appended to v
    (rowsum rides along as PSUM partition 64 of the ctx accumulation).
  - Matmuls of the same shape are batched into runs (PE shape switches
    measured ~0.9us each on TRN2).
"""

import numpy as np
import ml_dtypes

import concourse.bass as bass
import concourse.mybir as mybir
import concourse.tile as tile
from concourse import bacc
from concourse.bass_utils import run_bass_kernel_spmd

B, L, C, H, HD = 4, 2048, 1024, 16, 64
NCORES = 8
HPC = H // NCORES  # heads per core = 2
F32 = mybir.dt.float32
F32R = mybir.dt.float32r
BF16 = mybir.dt.bfloat16
EXP = mybir.ActivationFunctionType.Exp
LN = mybir.ActivationFunctionType.Ln

LCHUNK = 512          # token chunk for moving operands
NLC = L // LCHUNK     # 4
NKT = L // 128        # 16 k tiles per sequence
NCT = C // 128        # 8 contraction tiles for the projections


def build_kernel():
    nc = bacc.Bacc("TRN2", target_bir_lowering=False, debug=False,
                   num_devices=NCORES)

    xT = nc.dram_tensor("xT", [B, C, L], F32R, kind="ExternalInput")
    # wqkv[ci, j] = [128 c, 128 f] tile; j in (0=q both heads, 1=k, 2=v)
    wqkv = nc.dram_tensor("wqkv", [NCT, 3, 128, 128], F32R, kind="ExternalInput")
    bqkv = nc.dram_tensor("bqkv", [3, 128, 1], F32, kind="ExternalInput")
    # wo2: [128 c(2 heads), 1024 o]
    wo2 = nc.dram_tensor("wo2", [128, C], F32R, kind="ExternalInput")
    ident_d = nc.dram_tensor("ident_d", [128, 128], F32R, kind="ExternalInput")
    onesb_d = nc.dram_tensor("onesb_d", [128, 8], BF16, kind="ExternalInput")
    out = nc.dram_tensor("out", [B * L, C], F32, kind="ExternalOutput")

    with tile.TileContext(nc) as tc:
        kernel_body(nc, tc, xT, wqkv, bqkv, wo2, ident_d,
                    onesb_d, out)
    nc.compile()
    return nc


def kernel_body(nc, tc, xT, wqkv, bqkv, wo2, ident_d, onesb_d,
                out):
    from contextlib import ExitStack
    ctx = ExitStack()
    with ctx:
        consts = ctx.enter_context(tc.tile_pool(name="consts", bufs=1))
        xpool = ctx.enter_context(tc.tile_pool(name="xpool", bufs=14))
        qkvpool = ctx.enter_context(tc.tile_pool(name="qkvpool", bufs=2))
        vppool = ctx.enter_context(tc.tile_pool(name="vppool", bufs=24))
        epool = ctx.enter_context(tc.tile_pool(name="epool", bufs=12))
        cpool = ctx.enter_context(tc.tile_pool(name="cpool", bufs=2))
        spool = ctx.enter_context(tc.tile_pool(name="spool", bufs=2))
        opool = ctx.enter_context(tc.tile_pool(name="opool", bufs=3))
        # PSUM banks: s-tiles 2x2 + cacc 2 + general 2 = 8
        spsum = ctx.enter_context(tc.tile_pool(name="spsum", bufs=2,
                                               space="PSUM"))
        cpsum = ctx.enter_context(tc.tile_pool(name="cpsum", bufs=2,
                                               space="PSUM"))
        gpsum = ctx.enter_context(tc.tile_pool(name="gpsum", bufs=2,
                                               space="PSUM"))

        # ---- constants ----
        w_tiles = []
        for ci in range(NCT):
            row = []
            for j in range(3):
                t = consts.tile([128, 128], F32R, tag=f"w{ci}_{j}")
                nc.sync.dma_start(out=t, in_=wqkv[ci, j])
                row.append(t)
            w_tiles.append(row)
        b_tiles = []
        for j in range(3):
            t = consts.tile([128, 1], F32, tag=f"b{j}")
            nc.sync.dma_start(out=t, in_=bqkv[j])
            b_tiles.append(t)
        wo_t = consts.tile([128, C], F32R, tag="wo_t")
        nc.sync.dma_start(out=wo_t, in_=wo2[:])
        ident = consts.tile([128, 128], F32R, tag="ident")
        nc.sync.dma_start(out=ident, in_=ident_d[:])
        onesb = consts.tile([128, 8], BF16, tag="onesb")
        nc.sync.dma_start(out=onesb, in_=onesb_d[:])

        # ---- phase helpers (emitted in software-pipelined order below) ----
        def emit_qkv_loads(b, lc):
            ls = bass.ts(lc, LCHUNK)
            xts = []
            for ci in range(NCT):
                xt = xpool.tile([128, LCHUNK], F32R, tag="xt", name="xt")
                nc.sync.dma_start(out=xt, in_=xT[b, bass.ts(ci, 128), ls])
                xts.append(xt)
            return xts

        def emit_qkv_block(b, lc, qkvT, xts):
            ls = bass.ts(lc, LCHUNK)
            for j in range(3):
                p = gpsum.tile([128, LCHUNK], F32, tag="gpb", name="p")
                for ci in range(NCT):
                    nc.tensor.matmul(p, w_tiles[ci][j][:], xts[ci][:],
                                     start=(ci == 0), stop=(ci == NCT - 1))
                # PSUM -> SBUF with per-partition bias add. The q chunk is
                # split per head with the other head's rows zeroed so the
                # scores matmul can run K=128 (K-switches cost ~0.4us).
                if j == 0:
                    q0p, q1p = qkvT[0]
                    nc.vector.tensor_scalar_add(q0p[0:HD, ls], p[0:HD, :],
                                                b_tiles[0][0:HD])
                    nc.vector.tensor_scalar_mul(q0p[HD:128, ls],
                                                p[HD:128, :], 0.0)
                    nc.vector.tensor_scalar_add(q1p[HD:128, ls], p[HD:128, :],
                                                b_tiles[0][HD:128])
                    nc.vector.tensor_scalar_mul(q1p[0:HD, ls],
                                                p[0:HD, :], 0.0)
                else:
                    nc.vector.tensor_scalar_add(qkvT[j][:, ls], p,
                                                b_tiles[j][:])

        def emit_vplus(qkvT):
            # v -> token-major bf16 tiles [128 l, v_h0 | 1 | v_h1 | 1]
            vplus = []
            for t in range(NKT):
                tp = gpsum.tile([128, 128], F32R, tag="gpb", name="tp")
                nc.tensor.transpose(tp, qkvT[2][:, bass.ts(t, 128)], ident[:])
                vp = vppool.tile([128, 2 * HD + 2], BF16, tag="vp", name="vp")
                nc.vector.tensor_copy(vp[:, 0:HD], tp[:, 0:HD])
                nc.vector.tensor_copy(vp[:, HD + 1:2 * HD + 1], tp[:, HD:2 * HD])
                nc.vector.tensor_copy(vp[:, HD:HD + 1], onesb[:, 0:1])
                nc.vector.tensor_copy(vp[:, 2 * HD + 1:2 * HD + 2], onesb[:, 0:1])
                vplus.append(vp)
            return vplus

        def emit_attn_core(h, qc, qkvT, vplus):
            # one head, one 1024-wide q chunk; returns pending normalize work
            vsl = slice(h * (HD + 1), (h + 1) * (HD + 1))
            q0 = qc * 1024
            caccs = [cpsum.tile([HD + 1, LCHUNK], F32, tag="cpb",
                                name=f"cacc{half}")
                     for half in range(2)]
            evec = []
            qhp = qkvT[0][h]
            for i in range(NKT):
                s = spsum.tile([128, 2 * LCHUNK], F32, tag="spb", name="s")
                for half in range(2):
                    nc.tensor.matmul(
                        s[:, bass.ts(half, LCHUNK)],
                        qkvT[1][:, bass.ts(i, 128)],
                        qhp[:, bass.ds(q0 + half * LCHUNK, LCHUNK)],
                        start=True, stop=True)
                e = epool.tile([128, 2 * LCHUNK], BF16, tag="e", name="e")
                nc.scalar.activation(e, s, EXP, scale=0.125)
                evec.append(e)
            for i in range(NKT):
                for half in range(2):
                    nc.tensor.matmul(
                        caccs[half],
                        vplus[i][:, vsl],
                        evec[i][:, bass.ts(half, LCHUNK)],
                        start=(i == 0), stop=(i == NKT - 1))
            # copy cacc out immediately (frees the PSUM banks; rowsum row
            # lands on partition 0); the 1/Z chain is emitted later, batched
            # across chunks to avoid ACT Exp<->Ln table reloads (1.3us each)
            pend = []
            for half in range(2):
                cacc = caccs[half]
                qs = bass.ds(q0 + half * LCHUNK, LCHUNK)
                csb = spool.tile([HD, LCHUNK], F32, tag="csb", name="csb", bufs=5)
                nc.vector.tensor_copy(csb, cacc[0:HD, :])
                z0 = spool.tile([1, LCHUNK], F32, tag="z0", name="z0", bufs=5)
                nc.vector.tensor_copy(z0[0:1, :], cacc[HD:HD + 1, :])
                pend.append((h, qs, csb, z0))
            return pend

        def emit_normalize(pend, ctxT2):
            # batched: all Ln, then all Exp (one ACT table swap per group),
            # then GpSimd broadcasts and DVE multiplies
            lns = []
            for (h, qs, csb, z0) in pend:
                lnz = spool.tile([1, LCHUNK], F32, tag="lnz", name="lnz", bufs=5)
                nc.scalar.activation(lnz[0:1, :], z0[0:1, :], LN)
                lns.append(lnz)
            rts = []
            for lnz in lns:
                rt0 = spool.tile([1, LCHUNK], F32, tag="rt0", name="rt0", bufs=5)
                nc.scalar.activation(rt0[0:1, :], lnz[0:1, :], EXP, scale=-1.0)
                rts.append(rt0)
            for (h, qs, csb, z0), rt0 in zip(pend, rts):
                zs = spool.tile([HD, LCHUNK], F32, tag="zs", name="zs", bufs=3)
                nc.gpsimd.partition_broadcast(zs[0:HD, :], rt0[0:1, :])
                nc.vector.tensor_mul(ctxT2[h * HD:h * HD + HD, qs],
                                     csb[0:HD, :], zs)

        def emit_outproj(b, ctxT2, trange):
            for t in trange:
                rows = bass.ds(b * L + t * 128, 128)
                for oc in range(C // 512):
                    os_ = bass.ts(oc, 512)
                    o = gpsum.tile([128, 512], F32, tag="gpb", name="o")
                    nc.tensor.matmul(o, ctxT2[:, bass.ts(t, 128)],
                                     wo_t[:, os_], start=True, stop=True)
                    ot = opool.tile([128, 512], F32, tag="ot", name="ot")
                    nc.vector.tensor_copy(ot, o)
                    nc.sync.dma_start(out=out[rows, os_], in_=ot)

        # ---- software-pipelined emission ----
        # Interleave next batch's qkv blocks between attention chunks so the
        # PE's in-order queue always has dense, ready work behind any stall.
        def new_qkvT():
            qp = tuple(qkvpool.tile([128, L], F32R, tag=f"q{h}p",
                                    name=f"q{h}p") for h in range(HPC))
            kc = qkvpool.tile([128, L], F32R, tag="kc", name="kc")
            vc = qkvpool.tile([128, L], F32R, tag="vc", name="vc")
            return [qp, kc, vc]

        qkvT = new_qkvT()
        for lc in range(NLC):
            xts = emit_qkv_loads(0, lc)
            emit_qkv_block(0, lc, qkvT, xts)
        vplus = emit_vplus(qkvT)
        for b in range(B):
            ctxT2 = cpool.tile([128, L], F32R, tag="ctxT2", name="ctxT2")
            nxt = new_qkvT() if b + 1 < B else None
            # column-half order: both heads of q-chunk 0, then q-chunk 1, so
            # outproj l-tiles can start after the first half completes
            chunks = [(0, 0), (1, 0), (0, 1), (1, 1)]
            loads = {}
            if nxt is not None:
                loads[0] = emit_qkv_loads(b + 1, 0)
                loads[1] = emit_qkv_loads(b + 1, 1)
            pend = []
            for k, (h, qc) in enumerate(chunks):
                pend += emit_attn_core(h, qc, qkvT, vplus)
                if nxt is not None:
                    if k + 2 < NLC:
                        loads[k + 2] = emit_qkv_loads(b + 1, k + 2)
                    emit_qkv_block(b + 1, k, nxt, loads.pop(k))
                if k == 1:
                    emit_normalize(pend, ctxT2)
                    pend = []
                    emit_outproj(b, ctxT2, range(0, NKT // 2))
            emit_normalize(pend, ctxT2)
            if nxt is not None:
                nxt_vplus = emit_vplus(nxt)
            emit_outproj(b, ctxT2, range(NKT // 2, NKT))
            if nxt is not None:
                qkvT, vplus = nxt, nxt_vplus


_NC_CACHE = None


def get_nc():
    global _NC_CACHE
    if _NC_CACHE is None:
        _NC_CACHE = build_kernel()
    return _NC_CACHE


def prepare_in_maps(x, W_qkv, b_qkv, W_out, b_out):
    x = np.ascontiguousarray(np.asarray(x, np.float32))
    W_qkv = np.asarray(W_qkv, np.float32)
    b_qkv = np.asarray(b_qkv, np.float32)
    W_out = np.asarray(W_out, np.float32)
    b_out = np.asarray(b_out, np.float32)

    xT = np.ascontiguousarray(x.transpose(0, 2, 1))  # [B, C, L]

    in_maps = []
    for core in range(NCORES):
        h0 = HPC * core
        # per-head channel rows in W_qkv: q = h*192..+64, k = +64, v = +128
        qrows = [np.arange(h * 192, h * 192 + 64) for h in (h0, h0 + 1)]
        krows = [q + 64 for q in qrows]
        vrows = [q + 128 for q in qrows]
        fq = np.concatenate(qrows)
        fk = np.concatenate(krows)
        fv = np.concatenate(vrows)
        # wqkv tiles: [ci, j, 128 c, 128 f]
        wt = np.empty((NCT, 3, 128, 128), np.float32)
        for j, rows in enumerate((fq, fk, fv)):
            wT = np.ascontiguousarray(W_qkv[rows].T)  # [1024 c, 128 f]
            wt[:, j] = wT.reshape(NCT, 128, 128)
        bq = np.stack([b_qkv[fq], b_qkv[fk], b_qkv[fv]])[..., None]  # [3,128,1]
        # wo2 = [128 c, 1024 o]: rows 0:64 h0 ctx channels, 64:128 h1
        wo2 = np.concatenate([
            np.ascontiguousarray(W_out[:, (h0 + h) * HD:(h0 + h + 1) * HD].T)
            for h in range(HPC)
        ], axis=0)
        in_maps.append({
            "xT": xT,
            "wqkv": wt,
            "bqkv": np.ascontiguousarray(bq),
            "wo2": np.ascontiguousarray(wo2, dtype=np.float32),
            "ident_d": np.eye(128, dtype=np.float32),
            "onesb_d": np.ones((128, 8), ml_dtypes.bfloat16),
        })
    return in_maps


def kernel(x, W_qkv, b_qkv, W_out, b_out):
    in_maps = prepare_in_maps(x, W_qkv, b_qkv, W_out, b_out)
    res = run_bass_kernel_spmd(get_nc(), in_maps, core_ids=list(range(NCORES)))
    acc = np.zeros((B * L, C), np.float64)
    for core_out in res.results:
        acc += core_out["out"]
    acc += np.asarray(b_out, np.float64)[None, :]
    return acc.reshape(B, L, C).astype(np.float32)


if __name__ == "__main__":
    rng = np.random.default_rng(0)
    ins = {
        "x": rng.standard_normal((B, L, C)).astype(np.float32),
        "W_qkv": rng.uniform(-1 / 32, 1 / 32, (3 * C, C)).astype(np.float32),
        "b_qkv": rng.uniform(-1 / 32, 1 / 32, (3 * C,)).astype(np.float32),
        "W_out": rng.uniform(-1 / 32, 1 / 32, (C, C)).astype(np.float32),
        "b_out": rng.uniform(-1 / 32, 1 / 32, (C,)).astype(np.float32),
    }
    o = kernel(**ins)
    print(o.shape, o.dtype)
